# revision 3
# baseline (speedup 1.0000x reference)
"""GATv2Conv(64, 1024, heads=16) + Linear(16384, 20) Trainium2 kernel, v2.

Channel-major ("transposed") architecture. Per core (512 dst nodes, ~2560
incoming edges after balancing):

  logits:  a.lrelu(v) decomposes as  L = (sl[src]+sr[dst]) + 0.8*R,
           R[e,h] = sum_c att_hc * relu(-v_c),  v = x_l[src]+x_r[dst]
    - PE produces Y^T = [W_l;W_r]^T [x_src|x_dst] in ch-major [128ch, e]
      chunks via fp8 DoubleRow matmuls (K=2x64 features)
    - Act (Relu, negative per-partition |att| scale) and DVE (min(v,0)*-|att|)
      split the PSUM->SBUF pass producing u = |att|*relu(-Y) in fp8
    - PE reduces over channel partitions with a +-1 selector via fp8
      DoubleRow matmuls accumulating T^T [16h, e]
  softmax: P^T = exp(c*T) * exp(base) (host-baked exp(base)); DVE stream-
           transpose to edge-major; segment sum/denievne broadcast via S01T/S01
           matmuls as before
  agg:     G^T[64f, (h,d)] = X_src^T @ (alpha*S01T) per tile;
           aggT[128c, (t,d)] = W_l_chunk^T @ G^T (4 tiles batched, N=512);
           relu+bias split Act/DVE; z^T[20, (t,d)] = Wout_chunk^T @ reluT
  output:  z^T [20, 512] per core, host transposes + un-permutes + b_out.

Host balances nodes across 128-node dst-tiles so each tile has ~640 incoming
edges (5 subtiles of 128); subtile counts are baked per compile.
"""

import numpy as np

N_NODES = 4096
N_EDGES = 16384
F_IN = 64
H = 16
C = 1024
HC = H * C
N_CLASS = 20
N_CORES = 8
NODES_PER_CORE = N_NODES // N_CORES  # 512
TILES_PER_CORE = 4
NT = 128
NG = HC // 128  # 128 channel groups
NPAIR = NG // 2  # 64 group pairs
SA = 8.0  # fp8 absY scale
SW8 = 256.0  # fp8 lo-produce weight scale
CT = 0.8 / SA  # logit = base + CT * T
NGH = NG // 2  # bf16 (hi-|att|) channel groups
NGL = NG // 2  # fp8-DR (lo-|att|) channel groups
NEG_SLOPE = 0.2

_CACHE = {}


def _build_nc(S, act_mask, relu_act_mask):
    """S: per-tile-slot subtile counts (len 4, same for all cores).
    act_mask[g]: True -> Act handles relu-scale of channel group g.
    relu_act_mask[kk]: True -> Act handles relu of agg chunk kk."""
    import concourse.bacc as bacc
    import concourse.bass as bass
    import concourse.mybir as mybir
    import concourse.tile as tile

    f32 = mybir.dt.float32
    bf16 = mybir.dt.bfloat16
    fp8 = mybir.dt.float8e4
    AF = mybir.ActivationFunctionType
    OP = mybir.AluOpType
    DR = mybir.MatmulPerfMode.DoubleRow

    NSUB = sum(S)
    EPC = NSUB * 128
    B = [0]
    for s in S:
        B.append(B[-1] + s)  # subtile prefix offsets per tile

    # edge chunks of up to 4 subtiles (512 edges)
    chunks = []
    sub0 = 0
    while sub0 < NSUB:
        n = min(4, NSUB - sub0)
        chunks.append((sub0 * 128, n * 128))
        sub0 += n

    nc = bacc.Bacc("TRN2", target_bir_lowering=False)

    d_xcat = nc.dram_tensor("xcat", [128, EPC], bf16, kind="ExternalInput")
    d_xcat8 = nc.dram_tensor("xcat8", [64, 2, EPC], fp8, kind="ExternalInput")
    d_wcat = nc.dram_tensor("wcat", [128, NGH, 128], bf16, kind="ExternalInput")
    d_wcat8 = nc.dram_tensor("wcat8", [64, 2, NGL, 128], fp8, kind="ExternalInput")
    d_sel = nc.dram_tensor("sel", [128, 2, NPAIR, H], fp8, kind="ExternalInput")
    d_ebT = nc.dram_tensor("ebT", [32, EPC], bf16, kind="ExternalInput")
    d_s01t = nc.dram_tensor("s01t", [128, NSUB, NT], bf16, kind="ExternalInput")
    d_s01 = nc.dram_tensor("s01", [128, EPC], bf16, kind="ExternalInput")
    d_xsrc = nc.dram_tensor("xsrc", [128, NSUB, F_IN], bf16, kind="ExternalInput")
    d_wl = nc.dram_tensor("wl", [64, NG, 128], bf16, kind="ExternalInput")
    d_wout = nc.dram_tensor("wout", [128, NG, N_CLASS], bf16, kind="ExternalInput")
    d_bias = nc.dram_tensor("bias", [128, NG], f32, kind="ExternalInput")
    d_z = nc.dram_tensor("z", [N_CLASS, NODES_PER_CORE], f32, kind="ExternalOutput")

    with tile.TileContext(nc) as tc:
        with (
            tc.tile_pool(name="const", bufs=1) as cpool,
            tc.tile_pool(name="absy", bufs=8) as aypool,
            tc.tile_pool(name="small", bufs=2) as smpool,
            tc.tile_pool(name="als", bufs=max(S)) as aspool,
            tc.tile_pool(name="relu", bufs=4) as rpool,
            tc.tile_pool(name="psY", bufs=2, space=bass.MemorySpace.PSUM) as psY,
            tc.tile_pool(name="psT", bufs=1, space=bass.MemorySpace.PSUM) as psT,
            tc.tile_pool(name="psM", bufs=2, space=bass.MemorySpace.PSUM) as psM,
            tc.tile_pool(name="psZ", bufs=1, space=bass.MemorySpace.PSUM) as psZ,
        ):
            xcat = cpool.tile([128, EPC], bf16)
            xcat8 = cpool.tile([64, 2, EPC], fp8)
            wcat = cpool.tile([128, NGH, 128], bf16)
            wcat8 = cpool.tile([64, 2, NGL, 128], fp8)
            sel = cpool.tile([128, 2, NPAIR, H], fp8)
            ebT = cpool.tile([32, EPC], bf16)
            s01t = cpool.tile([128, NSUB, NT], bf16)
            s01 = cpool.tile([128, EPC], bf16)
            xsrc = cpool.tile([128, NSUB, F_IN], bf16)
            wl = cpool.tile([64, NG, 128], bf16)
            wout = cpool.tile([128, NG, N_CLASS], bf16)
            bias = cpool.tile([128, NG], f32)
            for t_, d_ in [
                (xcat, d_xcat), (xcat8, d_xcat8), (wcat, d_wcat),
                (wcat8, d_wcat8), (sel, d_sel),
                (ebT, d_ebT), (s01t, d_s01t), (s01, d_s01), (xsrc, d_xsrc),
                (wl, d_wl), (wout, d_wout), (bias, d_bias),
            ]:
                nc.sync.dma_start(t_[:], d_[:])

            ET = cpool.tile([32, EPC], bf16)
            PT = cpool.tile([32, NSUB, NT], bf16)
            P_em = cpool.tile([128, NSUB, 32], bf16)
            GT = cpool.tile([64, TILES_PER_CORE, H, NT], bf16)
            nc.vector.memset(ET[:], 0.0)

            # ---------------- phase Y: logits, transposed ----------------
            SEL_LAG = 5

            def phase_y_chunk(ci):
                e0, ecw = chunks[ci]
                tps = psT.tile([16, 512], f32, tag="T")
                pend = []

                def emit_sel(m, ay):
                    nc.tensor.matmul(
                        tps[:, :ecw],
                        sel[:, :, m, :],
                        ay[:, :, :ecw],
                        perf_mode=DR,
                        start=(m == 0),
                        stop=(m == NPAIR - 1),
                    )

                for m in range(NPAIR):
                    is8 = (m % 4) >= 2  # pairs 2,3 of each head are fp8-DR
                    ay = aypool.tile([128, 2, 512], fp8, tag="ay")
                    y = psY.tile([128, 2, 512], f32, tag="y")
                    for i in range(2):
                        g = 2 * m + i
                        hh, j = g // 8, g % 8
                        if is8:
                            nc.tensor.matmul(
                                y[:, i, :ecw],
                                wcat8[:, :, hh * 4 + (j - 4), :],
                                xcat8[:, :, e0 : e0 + ecw],
                                perf_mode=DR,
                            )
                        else:
                            nc.tensor.matmul(
                                y[:, i, :ecw],
                                wcat[:, hh * 4 + j, :],
                                xcat[:, e0 : e0 + ecw],
                            )
                    sc = -SA / SW8 if is8 else -SA
                    if act_mask[m]:
                        nc.scalar.activation(
                            ay[:, :, :ecw], y[:, :, :ecw], AF.Relu, scale=sc
                        )
                    else:
                        nc.vector.tensor_scalar(
                            out=ay[:, :, :ecw], in0=y[:, :, :ecw],
                            scalar1=0.0, scalar2=sc,
                            op0=OP.min, op1=OP.mult,
                        )
                    pend.append((m, ay))
                    if len(pend) > SEL_LAG:
                        emit_sel(*pend.pop(0))
                while pend:
                    emit_sel(*pend.pop(0))
                # P^T = exp(CT*T) * ebT
                nc.scalar.activation(
                    ET[0:16, e0 : e0 + ecw], tps[:, :ecw], AF.Exp, scale=CT
                )
                nc.vector.tensor_tensor(
                    out=PT.rearrange("p s n -> p (s n)")[:, e0 : e0 + ecw],
                    in0=ET[:, e0 : e0 + ecw],
                    in1=ebT[:, e0 : e0 + ecw],
                    op=OP.mult,
                )
                # edge-major P for this chunk's subtiles
                s0, ns = e0 // 128, ecw // 128
                for r in range(4):
                    nc.vector.transpose(
                        P_em[32 * r : 32 * (r + 1), s0 : s0 + ns, :],
                        PT[:, s0 : s0 + ns, 32 * r : 32 * (r + 1)],
                    )

            # ---------------- per-tile softmax + G ----------------
            def tile_phase(t):
                St = S[t]
                ss = psM.tile([128, H], f32, tag="misc")
                for s in range(St):
                    nc.tensor.matmul(
                        ss[:],
                        s01t[:, B[t] + s, :],
                        P_em[:, B[t] + s, 0:H],
                        start=(s == 0),
                        stop=(s == St - 1),
                    )
                ssb = smpool.tile([128, H], bf16, tag="ssb")
                nc.scalar.copy(ssb[:], ss[:])

                den = psM.tile([128, St, H], f32, tag="misc")
                for s in range(St):
                    nc.tensor.matmul(
                        den[:, s, :],
                        s01[:, (B[t] + s) * 128 : (B[t] + s + 1) * 128],
                        ssb[:],
                    )
                rec = smpool.tile([128, St, H], f32, tag="rec")
                nc.vector.reciprocal(rec[:], den[:])
                alpha = smpool.tile([128, St, H], f32, tag="alpha")
                nc.vector.tensor_tensor(
                    out=alpha[:], in0=P_em[:, B[t] : B[t] + St, 0:H], in1=rec[:],
                    op=OP.mult,
                )

                ass = []
                for s in range(St):
                    aS = aspool.tile([128, H, NT], bf16, tag="as")
                    ass.append(aS)
                    for h in range(H):
                        nc.vector.tensor_scalar_mul(
                            aS[:, h, :],
                            s01t[:, B[t] + s, :],
                            alpha[:, s, h : h + 1],
                        )
                for q in range(4):
                    gq = psM.tile([64, 4 * NT], f32, tag="misc")
                    for s in range(St):
                        nc.tensor.matmul(
                            gq[:],
                            xsrc[:, B[t] + s, :],
                            ass[s][:, 4 * q : 4 * (q + 1), :],
                            start=(s == 0),
                            stop=(s == St - 1),
                        )
                    nc.scalar.copy(GT[:, t, 4 * q : 4 * (q + 1), :], gq[:])

            # interleaved emission: chunks, with tile phases trailing
            n_ch = len(chunks)
            done_subs = 0
            next_tile = 0
            for ci in range(n_ch):
                phase_y_chunk(ci)
                done_subs += chunks[ci][1] // 128
                while next_tile < TILES_PER_CORE and done_subs >= B[next_tile + 1] + 4:
                    tile_phase(next_tile)
                    next_tile += 1
            while next_tile < TILES_PER_CORE:
                tile_phase(next_tile)
                next_tile += 1

            # ---------------- aggT + relu + z^T ----------------
            zps = psZ.tile([N_CLASS, TILES_PER_CORE * NT], f32, tag="z")
            zpend = []

            def emit_z(kk, reluT):
                nc.tensor.matmul(
                    zps[:],
                    wout[:, kk, :],
                    reluT[:],
                    start=(kk == 0),
                    stop=(kk == NG - 1),
                )

            for kk in range(NG):
                h = kk // 8
                aps = psM.tile([128, TILES_PER_CORE * NT], f32, tag="misc")
                nc.tensor.matmul(aps[:], wl[:, kk, :], GT[:, :, h, :])
                reluT = rpool.tile([128, TILES_PER_CORE * NT], bf16, tag="r")
                if relu_act_mask[kk]:
                    nc.scalar.activation(
                        reluT[:], aps[:], AF.Relu, bias=bias[:, kk : kk + 1]
                    )
                else:
                    nc.vector.tensor_scalar(
                        out=reluT[:], in0=aps[:],
                        scalar1=bias[:, kk : kk + 1], scalar2=0.0,
                        op0=OP.add, op1=OP.max,
                    )
                zpend.append((kk, reluT))
                if len(zpend) > 3:
                    emit_z(*zpend.pop(0))
            while zpend:
                emit_z(*zpend.pop(0))
            z_sb = cpool.tile([N_CLASS, TILES_PER_CORE * NT], f32)
            nc.vector.tensor_copy(z_sb[:], zps[:])
            nc.sync.dma_start(d_z[:], z_sb[:])

    nc.compile()
    return nc


def _balance_tiles(deg):
    """Assign nodes to 32 tiles of 128 nodes, equalizing per-tile edge counts.
    Returns perm: perm[i] = original node id at permuted position i."""
    n_tiles = N_NODES // NT
    target = int(deg.sum()) // n_tiles
    order = np.argsort(-deg, kind="stable")
    tile_nodes = [[] for _ in range(n_tiles)]
    tile_edges = np.zeros(n_tiles, np.int64)
    for n in order:
        free = np.array([len(tile_nodes[t]) < NT for t in range(n_tiles)])
        cand = np.where(free)[0]
        t = cand[np.argmin(tile_edges[cand])]
        tile_nodes[t].append(n)
        tile_edges[t] += deg[n]
    # swap refinement toward exactly `target` per tile
    for _ in range(6):
        over = [t for t in range(n_tiles) if tile_edges[t] > target]
        under = [t for t in range(n_tiles) if tile_edges[t] < target]
        if not over:
            break
        for to in over:
            for tu in under:
                excess = tile_edges[to] - target
                deficit = target - tile_edges[tu]
                d = min(excess, deficit)
                if d <= 0:
                    continue
                # find node pair (a in to, b in tu) with deg[a]-deg[b] == d
                degs_o = {deg[a]: a for a in tile_nodes[to]}
                degs_u = {deg[b]: b for b in tile_nodes[tu]}
                done = False
                for da, a in sorted(degs_o.items(), reverse=True):
                    db = da - d
                    if db in degs_u:
                        b = degs_u[db]
                        tile_nodes[to].remove(a)
                        tile_nodes[tu].remove(b)
                        tile_nodes[to].append(b)
                        tile_nodes[tu].append(a)
                        tile_edges[to] -= d
                        tile_edges[tu] += d
                        done = True
                        break
                if done and tile_edges[to] <= target:
                    break
    perm = np.concatenate([np.array(tile_nodes[t], np.int64) for t in range(n_tiles)])
    return perm, tile_edges


def _prep_inputs(x, edge_index, W_l, W_r, att, bias_gat, W_out, b_out):
    import ml_dtypes

    bf16 = ml_dtypes.bfloat16
    fp8 = ml_dtypes.float8_e4m3
    x = np.asarray(x, np.float32)
    W_l = np.asarray(W_l, np.float32)
    W_r = np.asarray(W_r, np.float32)
    att = np.asarray(att, np.float32).reshape(HC)
    bias_gat = np.asarray(bias_gat, np.float32)
    W_out = np.asarray(W_out, np.float32)

    src = np.concatenate([np.asarray(edge_index[0]), np.arange(N_NODES)]).astype(
        np.int64
    )
    dst = np.concatenate([np.asarray(edge_index[1]), np.arange(N_NODES)]).astype(
        np.int64
    )
    E_tot = len(src)

    deg = np.bincount(dst, minlength=N_NODES)
    perm, tile_edges = _balance_tiles(deg)
    # node -> (tile, position-in-tile); new node id = position in perm
    newid = np.empty(N_NODES, np.int64)
    newid[perm] = np.arange(N_NODES)

    # per-tile subtile counts (same structure for every core): sort each
    # core's 4 tiles by descending subtile need, take slot-wise max
    n_tiles = N_NODES // NT
    subs = np.array([(tile_edges[t] + NT - 1) // NT for t in range(n_tiles)])
    subs_by_core = subs.reshape(N_CORES, TILES_PER_CORE)
    # reorder tiles within each core desc so slot-wise max is tight
    tile_order = np.argsort(-subs_by_core, axis=1, kind="stable")
    S = np.max(np.sort(subs_by_core, axis=1)[:, ::-1], axis=0).tolist()
    NSUB = sum(S)
    EPC = NSUB * 128

    # per-head channel order: |att| descending; first 512 -> bf16 produce,
    # last 512 -> fp8 DoubleRow produce. T-sum is channel-order invariant.
    ord_ch = np.argsort(-np.abs(att).reshape(H, C), axis=1, kind="stable")
    ch_perm = (ord_ch + np.arange(H)[:, None] * C).reshape(-1)  # new->orig hc
    # group g (of permuted order) covers perm channels g*128..+128; per head
    # groups h*8+0..3 are hi (bf16), +4..7 are lo (fp8)
    Wla = np.vstack([W_l, W_r]) * np.abs(att)[None, :]  # [128f, HC]
    Wp = Wla[:, ch_perm].reshape(128, H, 8, 128)
    hi = np.ascontiguousarray(
        Wp[:, :, 0:4, :].reshape(128, NGH, 128)
    )
    lo = np.ascontiguousarray(
        Wp[:, :, 4:8, :].reshape(128, NGL, 128)
    ) * SW8
    wcat_b = hi.astype(bf16)
    wcat8_b = np.ascontiguousarray(
        lo.reshape(2, 64, NGL, 128).transpose(1, 0, 2, 3)
    ).astype(fp8)  # [f64, ktile(l/r), g, c]
    sgn_p = np.sign(att)[ch_perm].reshape(H, 8, 128)
    selv = np.zeros((128, 2, NPAIR, H), np.float32)
    for g in range(NG):
        hh, j = g // 8, g % 8
        selv[:, g % 2, g // 2, hh] = sgn_p[hh, j]
    sel8 = selv.astype(fp8)

    wl_b = np.ascontiguousarray(
        W_l.reshape(64, NG, 128)
    ).astype(bf16)  # [f, g, c]
    wout_b = np.ascontiguousarray(
        W_out.reshape(NG, 128, N_CLASS).transpose(1, 0, 2)
    ).astype(bf16)  # [p, g, 20]
    bias_sb = np.ascontiguousarray(bias_gat.reshape(NG, 128).T).astype(np.float32)

    # per-node logit scalars
    ul = np.einsum("fhc,hc->fh", W_l.reshape(F_IN, H, C), att.reshape(H, C))
    ur = np.einsum("fhc,hc->fh", W_r.reshape(F_IN, H, C), att.reshape(H, C))
    sl = x @ ul
    sr = x @ ur

    # stable sort edges by destination tile
    dtile = newid[dst] // NT
    order = np.argsort(dtile, kind="stable")
    src_s, dst_s = src[order], dst[order]
    dtile_s = dtile[order]

    in_maps = []
    for core in range(N_CORES):
        xcat = np.zeros((128, EPC), np.float32)
        xcat8 = np.zeros((64, 2, EPC), np.float32)
        ebT = np.zeros((32, EPC), np.float32)
        s01t = np.zeros((128, NSUB, NT), np.float32)
        s01 = np.zeros((NT, EPC), np.float32)
        xsrc = np.zeros((128, NSUB, F_IN), np.float32)
        b0 = 0
        for slot in range(TILES_PER_CORE):
            gt = core * TILES_PER_CORE + tile_order[core][slot]
            idx = np.nonzero(dtile_s == gt)[0]
            ne = len(idx)
            assert ne <= S[slot] * 128, f"tile {gt}: {ne} > {S[slot] * 128}"
            es, ed = src_s[idx], dst_s[idx]
            eslot = np.arange(ne)
            e_abs = b0 * 128 + eslot
            p = eslot % 128
            s_sub = b0 + eslot // 128
            xcat[0:64, e_abs] = x[es].T
            xcat[64:128, e_abs] = x[ed].T
            xcat8[:, 0, e_abs] = x[es].T
            xcat8[:, 1, e_abs] = x[ed].T
            ebT[0:16, e_abs] = np.exp(sl[es] + sr[ed]).T
            ldst = newid[ed] - gt * NT
            s01t[p, s_sub, ldst] = 1.0
            s01[ldst, e_abs] = 1.0
            xsrc[p, s_sub, :] = x[es]
            b0 += S[slot]
        pad = s01.sum(axis=0) == 0.0
        s01[0, pad] = 1.0
        in_maps.append(
            {
                "xcat": xcat.astype(bf16),
                "xcat8": xcat8.astype(fp8),
                "wcat": wcat_b,
                "wcat8": wcat8_b,
                "sel": sel8,
                "ebT": ebT.astype(bf16),
                "s01t": s01t.astype(bf16),
                "s01": np.ascontiguousarray(s01).astype(bf16),
                "xsrc": xsrc.astype(bf16),
                "wl": wl_b,
                "wout": wout_b,
                "bias": bias_sb,
            }
        )
    meta = {"S": S, "perm": perm, "tile_order": tile_order}
    return in_maps, meta


def kernel(**inputs):
    from concourse.bass_utils import run_bass_kernel_spmd

    in_maps, meta = _prep_inputs(**inputs)
    S = meta["S"]
    key = tuple(S)
    if key not in _CACHE:
        act_mask = [m % 2 == 0 for m in range(NPAIR)]
        relu_act_mask = [True for _ in range(NG)]
        _CACHE[key] = _build_nc(S, act_mask, relu_act_mask)
    nc = _CACHE[key]

    res = run_bass_kernel_spmd(nc, in_maps, list(range(N_CORES)))
    b_out = np.asarray(inputs["b_out"], np.float32)
    perm = meta["perm"]
    tile_order = meta["tile_order"]
    z = np.empty((N_NODES, N_CLASS), np.float32)
    for core in range(N_CORES):
        zc = np.asarray(res.results[core]["z"], np.float32)  # [20, 512]
        for slot in range(TILES_PER_CORE):
            gt = core * TILES_PER_CORE + tile_order[core][slot]
            orig_nodes = perm[gt * NT : (gt + 1) * NT]
            z[orig_nodes] = zc[:, slot * NT : (slot + 1) * NT].T
    return z + b_out


# revision 8
# speedup vs baseline: 1.0049x; 1.0049x over previous
"""GATv2Conv(64, 1024, heads=16) + Linear(16384, 20) Trainium2 kernel, v2.

Channel-major ("transposed") architecture, dst-sharded over 8 cores. Per core
(512 dst nodes, ~2560 incoming edges after host-side degree balancing):

  logits:  a.lrelu(v) decomposes as  L = (sl[src]+sr[dst]) + 0.8*R,
           R[e,h] = sum_c att_hc * relu(-v_c),  v = x_l[src]+x_r[dst];
           the linear term is host-baked into exp(base) per edge.
    - PE produces Y'^T = ([W_l;W_r] o |att|)^T [x_src|x_dst] in channel-major
      [128ch, 512e] PSUM chunks. Per head, channels are sorted by |att|:
      the top half runs bf16 (K=128), the bottom half (carrying ~18% of the
      logit error weight) runs fp8 DoubleRow (K=2x64).
    - Act (Relu with negative scale) and DVE (min(v,0)*-SA) split the
      PSUM->SBUF pass producing u = SA*|att|*relu(-Y) in fp8e4, one
      instruction per group pair [128, 2, 512].
    - PE reduces over channel partitions with a +-sign(att) selector via fp8
      DoubleRow matmuls accumulating T^T [16h, e]; sel emission lags produce
      so the in-order PE queue never stalls on the abs-cast.
  softmax: P^T = exp(CT*T) * exp(base) (Act exp + DVE mult); DVE stream-
           transpose 32x32 blocks to edge-major; segment-sum and denominator
           broadcast via S01T/S01 one-hot matmuls per 128-dst-node tile
  agg:     G^T[64f, (h,d)] = X_src^T @ (alpha o S01T) per tile;
           aggT[128c, (4t,d)] = W_l_chunk^T @ G^T (4 tiles batched, N=512);
           relu+bias on Act; z^T[20, (t,d)] = Wout_chunk^T @ reluT (lagged)
  output:  z^T [20, 512] per core; host transposes, un-permutes, adds b_out.

Host balances nodes across 128-node dst-tiles (greedy + swap refinement) so
each tile has ~640 incoming edges (5 subtiles of 128); per-tile subtile
counts are baked into the compile (cached per structure).
"""

import numpy as np

N_NODES = 4096
N_EDGES = 16384
F_IN = 64
H = 16
C = 1024
HC = H * C
N_CLASS = 20
N_CORES = 8
NODES_PER_CORE = N_NODES // N_CORES  # 512
TILES_PER_CORE = 4
NT = 128
NG = HC // 128  # 128 channel groups
NPAIR = NG // 2  # 64 group pairs
SA = 8.0  # fp8 absY scale
SW8 = 256.0  # fp8 lo-produce weight scale
CT = 0.8 / SA  # logit = base + CT * T
NGH = NG // 2  # bf16 (hi-|att|) channel groups
NGL = NG // 2  # fp8-DR (lo-|att|) channel groups
NEG_SLOPE = 0.2

_CACHE = {}


def _build_nc(S, act_mask, relu_act_mask):
    """S: per-tile-slot subtile counts (len 4, same for all cores).
    act_mask[g]: True -> Act handles relu-scale of channel group g.
    relu_act_mask[kk]: True -> Act handles relu of agg chunk kk."""
    import concourse.bacc as bacc
    import concourse.bass as bass
    import concourse.mybir as mybir
    import concourse.tile as tile

    f32 = mybir.dt.float32
    bf16 = mybir.dt.bfloat16
    fp8 = mybir.dt.float8e4
    AF = mybir.ActivationFunctionType
    OP = mybir.AluOpType
    DR = mybir.MatmulPerfMode.DoubleRow

    NSUB = sum(S)
    EPC = NSUB * 128
    B = [0]
    for s in S:
        B.append(B[-1] + s)  # subtile prefix offsets per tile

    # edge chunks of up to 4 subtiles (512 edges)
    chunks = []
    sub0 = 0
    while sub0 < NSUB:
        n = min(4, NSUB - sub0)
        chunks.append((sub0 * 128, n * 128))
        sub0 += n

    nc = bacc.Bacc("TRN2", target_bir_lowering=False)

    d_xcat = nc.dram_tensor("xcat", [128, EPC], bf16, kind="ExternalInput")
    d_xcat8 = nc.dram_tensor("xcat8", [64, 2, EPC], fp8, kind="ExternalInput")
    d_wcat = nc.dram_tensor("wcat", [128, NGH, 128], bf16, kind="ExternalInput")
    d_wcat8 = nc.dram_tensor("wcat8", [64, 2, NGL, 128], fp8, kind="ExternalInput")
    d_sel = nc.dram_tensor("sel", [128, 2, NPAIR, H], fp8, kind="ExternalInput")
    d_ebT = nc.dram_tensor("ebT", [32, EPC], bf16, kind="ExternalInput")
    d_s01t = nc.dram_tensor("s01t", [128, NSUB, NT], bf16, kind="ExternalInput")
    d_s01 = nc.dram_tensor("s01", [128, EPC], bf16, kind="ExternalInput")
    d_xsrc = nc.dram_tensor("xsrc", [128, NSUB, F_IN], bf16, kind="ExternalInput")
    d_wl = nc.dram_tensor("wl", [64, NG, 128], bf16, kind="ExternalInput")
    d_wout = nc.dram_tensor("wout", [128, NG, N_CLASS], bf16, kind="ExternalInput")
    d_bias = nc.dram_tensor("bias", [128, NG], f32, kind="ExternalInput")
    d_z = nc.dram_tensor("z", [N_CLASS, NODES_PER_CORE], f32, kind="ExternalOutput")

    with tile.TileContext(nc) as tc:
        with (
            tc.tile_pool(name="const", bufs=1) as cpool,
            tc.tile_pool(name="absy", bufs=8) as aypool,
            tc.tile_pool(name="small", bufs=2) as smpool,
            tc.tile_pool(name="als", bufs=max(S)) as aspool,
            tc.tile_pool(name="relu", bufs=4) as rpool,
            tc.tile_pool(name="psY", bufs=2, space=bass.MemorySpace.PSUM) as psY,
            tc.tile_pool(name="psT", bufs=1, space=bass.MemorySpace.PSUM) as psT,
            tc.tile_pool(name="psM", bufs=2, space=bass.MemorySpace.PSUM) as psM,
            tc.tile_pool(name="psZ", bufs=1, space=bass.MemorySpace.PSUM) as psZ,
        ):
            xcat = cpool.tile([128, EPC], bf16)
            xcat8 = cpool.tile([64, 2, EPC], fp8)
            wcat = cpool.tile([128, NGH, 128], bf16)
            wcat8 = cpool.tile([64, 2, NGL, 128], fp8)
            sel = cpool.tile([128, 2, NPAIR, H], fp8)
            ebT = cpool.tile([32, EPC], bf16)
            s01t = cpool.tile([128, NSUB, NT], bf16)
            s01 = cpool.tile([128, EPC], bf16)
            xsrc = cpool.tile([128, NSUB, F_IN], bf16)
            wl = cpool.tile([64, NG, 128], bf16)
            wout = cpool.tile([128, NG, N_CLASS], bf16)
            bias = cpool.tile([128, NG], f32)
            for t_, d_ in [
                (xcat, d_xcat), (xcat8, d_xcat8), (wcat, d_wcat),
                (wcat8, d_wcat8), (sel, d_sel),
                (ebT, d_ebT), (s01t, d_s01t), (s01, d_s01), (xsrc, d_xsrc),
                (wl, d_wl), (wout, d_wout), (bias, d_bias),
            ]:
                nc.sync.dma_start(t_[:], d_[:])

            ET = cpool.tile([32, EPC], bf16)
            PT = cpool.tile([32, NSUB, NT], bf16)
            P_em = cpool.tile([128, NSUB, 32], bf16)
            GT = cpool.tile([64, TILES_PER_CORE, H, NT], bf16)
            nc.vector.memset(ET[:], 0.0)

            # ---------------- phase Y: logits, transposed ----------------
            SEL_LAG = 5

            def phase_y_chunk(ci):
                e0, ecw = chunks[ci]
                tps = psT.tile([16, 512], f32, tag="T")
                pend = []

                def emit_sel(m, ay):
                    nc.tensor.matmul(
                        tps[:, :ecw],
                        sel[:, :, m, :],
                        ay[:, :, :ecw],
                        perf_mode=DR,
                        start=(m == 0),
                        stop=(m == NPAIR - 1),
                    )

                for m in range(NPAIR):
                    is8 = (m % 4) >= 2  # pairs 2,3 of each head are fp8-DR
                    ay = aypool.tile([128, 2, 512], fp8, tag="ay")
                    y = psY.tile([128, 2, 512], f32, tag="y")
                    for i in range(2):
                        g = 2 * m + i
                        hh, j = g // 8, g % 8
                        if is8:
                            nc.tensor.matmul(
                                y[:, i, :ecw],
                                wcat8[:, :, hh * 4 + (j - 4), :],
                                xcat8[:, :, e0 : e0 + ecw],
                                perf_mode=DR,
                            )
                        else:
                            nc.tensor.matmul(
                                y[:, i, :ecw],
                                wcat[:, hh * 4 + j, :],
                                xcat[:, e0 : e0 + ecw],
                            )
                    sc = -SA / SW8 if is8 else -SA
                    if act_mask[m]:
                        nc.scalar.activation(
                            ay[:, :, :ecw], y[:, :, :ecw], AF.Relu, scale=sc
                        )
                    else:
                        nc.vector.tensor_scalar(
                            out=ay[:, :, :ecw], in0=y[:, :, :ecw],
                            scalar1=0.0, scalar2=sc,
                            op0=OP.min, op1=OP.mult,
                        )
                    pend.append((m, ay))
                    if len(pend) > SEL_LAG:
                        emit_sel(*pend.pop(0))
                while pend:
                    emit_sel(*pend.pop(0))
                # P^T = exp(CT*T) * ebT
                nc.scalar.activation(
                    ET[0:16, e0 : e0 + ecw], tps[:, :ecw], AF.Exp, scale=CT
                )
                nc.vector.tensor_tensor(
                    out=PT.rearrange("p s n -> p (s n)")[:, e0 : e0 + ecw],
                    in0=ET[:, e0 : e0 + ecw],
                    in1=ebT[:, e0 : e0 + ecw],
                    op=OP.mult,
                )
                # edge-major P for this chunk's subtiles
                s0, ns = e0 // 128, ecw // 128
                for r in range(4):
                    nc.vector.transpose(
                        P_em[32 * r : 32 * (r + 1), s0 : s0 + ns, :],
                        PT[:, s0 : s0 + ns, 32 * r : 32 * (r + 1)],
                    )

            # ---------------- per-tile softmax + G ----------------
            def tile_phase(t):
                St = S[t]
                ss = psM.tile([128, H], f32, tag="misc")
                for s in range(St):
                    nc.tensor.matmul(
                        ss[:],
                        s01t[:, B[t] + s, :],
                        P_em[:, B[t] + s, 0:H],
                        start=(s == 0),
                        stop=(s == St - 1),
                    )
                ssb = smpool.tile([128, H], bf16, tag="ssb")
                nc.scalar.copy(ssb[:], ss[:])

                den = psM.tile([128, St, H], f32, tag="misc")
                for s in range(St):
                    nc.tensor.matmul(
                        den[:, s, :],
                        s01[:, (B[t] + s) * 128 : (B[t] + s + 1) * 128],
                        ssb[:],
                    )
                rec = smpool.tile([128, St, H], f32, tag="rec")
                nc.vector.reciprocal(rec[:], den[:])
                alpha = smpool.tile([128, St, H], f32, tag="alpha")
                nc.vector.tensor_tensor(
                    out=alpha[:], in0=P_em[:, B[t] : B[t] + St, 0:H], in1=rec[:],
                    op=OP.mult,
                )

                ass = []
                for s in range(St):
                    aS = aspool.tile([128, H, NT], bf16, tag="as")
                    ass.append(aS)
                    for h in range(H):
                        nc.vector.tensor_scalar_mul(
                            aS[:, h, :],
                            s01t[:, B[t] + s, :],
                            alpha[:, s, h : h + 1],
                        )
                for q in range(4):
                    gq = psM.tile([64, 4 * NT], f32, tag="misc")
                    for s in range(St):
                        nc.tensor.matmul(
                            gq[:],
                            xsrc[:, B[t] + s, :],
                            ass[s][:, 4 * q : 4 * (q + 1), :],
                            start=(s == 0),
                            stop=(s == St - 1),
                        )
                    nc.scalar.copy(GT[:, t, 4 * q : 4 * (q + 1), :], gq[:])

            # interleaved emission: chunks, with tile phases trailing
            n_ch = len(chunks)
            done_subs = 0
            next_tile = 0
            for ci in range(n_ch):
                phase_y_chunk(ci)
                done_subs += chunks[ci][1] // 128
                while next_tile < TILES_PER_CORE and done_subs >= B[next_tile + 1] + 4:
                    tile_phase(next_tile)
                    next_tile += 1
            while next_tile < TILES_PER_CORE:
                tile_phase(next_tile)
                next_tile += 1

            # ---------------- aggT + relu + z^T ----------------
            zps = psZ.tile([N_CLASS, TILES_PER_CORE * NT], f32, tag="z")
            zpend = []

            def emit_z(kk, reluT):
                nc.tensor.matmul(
                    zps[:],
                    wout[:, kk, :],
                    reluT[:],
                    start=(kk == 0),
                    stop=(kk == NG - 1),
                )

            for kk in range(NG):
                h = kk // 8
                aps = psM.tile([128, TILES_PER_CORE * NT], f32, tag="misc")
                nc.tensor.matmul(aps[:], wl[:, kk, :], GT[:, :, h, :])
                reluT = rpool.tile([128, TILES_PER_CORE * NT], bf16, tag="r")
                if relu_act_mask[kk]:
                    nc.scalar.activation(
                        reluT[:], aps[:], AF.Relu, bias=bias[:, kk : kk + 1]
                    )
                else:
                    nc.vector.tensor_scalar(
                        out=reluT[:], in0=aps[:],
                        scalar1=bias[:, kk : kk + 1], scalar2=0.0,
                        op0=OP.add, op1=OP.max,
                    )
                zpend.append((kk, reluT))
                if len(zpend) > 3:
                    emit_z(*zpend.pop(0))
            while zpend:
                emit_z(*zpend.pop(0))
            z_sb = cpool.tile([N_CLASS, TILES_PER_CORE * NT], f32)
            nc.vector.tensor_copy(z_sb[:], zps[:])
            nc.sync.dma_start(d_z[:], z_sb[:])

    nc.compile()
    return nc


def _balance_tiles(deg):
    """Assign nodes to 32 tiles of 128 nodes, equalizing per-tile edge counts.
    Returns perm: perm[i] = original node id at permuted position i."""
    n_tiles = N_NODES // NT
    target = int(deg.sum()) // n_tiles
    order = np.argsort(-deg, kind="stable")
    tile_nodes = [[] for _ in range(n_tiles)]
    tile_edges = np.zeros(n_tiles, np.int64)
    for n in order:
        free = np.array([len(tile_nodes[t]) < NT for t in range(n_tiles)])
        cand = np.where(free)[0]
        t = cand[np.argmin(tile_edges[cand])]
        tile_nodes[t].append(n)
        tile_edges[t] += deg[n]
    # swap refinement toward exactly `target` per tile
    for _ in range(6):
        over = [t for t in range(n_tiles) if tile_edges[t] > target]
        under = [t for t in range(n_tiles) if tile_edges[t] < target]
        if not over:
            break
        for to in over:
            for tu in under:
                excess = tile_edges[to] - target
                deficit = target - tile_edges[tu]
                d = min(excess, deficit)
                if d <= 0:
                    continue
                # find node pair (a in to, b in tu) with deg[a]-deg[b] == d
                degs_o = {deg[a]: a for a in tile_nodes[to]}
                degs_u = {deg[b]: b for b in tile_nodes[tu]}
                done = False
                for da, a in sorted(degs_o.items(), reverse=True):
                    db = da - d
                    if db in degs_u:
                        b = degs_u[db]
                        tile_nodes[to].remove(a)
                        tile_nodes[tu].remove(b)
                        tile_nodes[to].append(b)
                        tile_nodes[tu].append(a)
                        tile_edges[to] -= d
                        tile_edges[tu] += d
                        done = True
                        break
                if done and tile_edges[to] <= target:
                    break
    perm = np.concatenate([np.array(tile_nodes[t], np.int64) for t in range(n_tiles)])
    return perm, tile_edges


def _prep_inputs(x, edge_index, W_l, W_r, att, bias_gat, W_out, b_out):
    import ml_dtypes

    bf16 = ml_dtypes.bfloat16
    fp8 = ml_dtypes.float8_e4m3
    x = np.asarray(x, np.float32)
    W_l = np.asarray(W_l, np.float32)
    W_r = np.asarray(W_r, np.float32)
    att = np.asarray(att, np.float32).reshape(HC)
    bias_gat = np.asarray(bias_gat, np.float32)
    W_out = np.asarray(W_out, np.float32)

    src = np.concatenate([np.asarray(edge_index[0]), np.arange(N_NODES)]).astype(
        np.int64
    )
    dst = np.concatenate([np.asarray(edge_index[1]), np.arange(N_NODES)]).astype(
        np.int64
    )
    E_tot = len(src)

    deg = np.bincount(dst, minlength=N_NODES)
    perm, tile_edges = _balance_tiles(deg)
    # node -> (tile, position-in-tile); new node id = position in perm
    newid = np.empty(N_NODES, np.int64)
    newid[perm] = np.arange(N_NODES)

    # per-tile subtile counts (same structure for every core): sort each
    # core's 4 tiles by descending subtile need, take slot-wise max
    n_tiles = N_NODES // NT
    subs = np.array([(tile_edges[t] + NT - 1) // NT for t in range(n_tiles)])
    subs_by_core = subs.reshape(N_CORES, TILES_PER_CORE)
    # reorder tiles within each core desc so slot-wise max is tight
    tile_order = np.argsort(-subs_by_core, axis=1, kind="stable")
    S = np.max(np.sort(subs_by_core, axis=1)[:, ::-1], axis=0).tolist()
    NSUB = sum(S)
    EPC = NSUB * 128

    # per-head channel order: |att| descending; first 512 -> bf16 produce,
    # last 512 -> fp8 DoubleRow produce. T-sum is channel-order invariant.
    ord_ch = np.argsort(-np.abs(att).reshape(H, C), axis=1, kind="stable")
    ch_perm = (ord_ch + np.arange(H)[:, None] * C).reshape(-1)  # new->orig hc
    # group g (of permuted order) covers perm channels g*128..+128; per head
    # groups h*8+0..3 are hi (bf16), +4..7 are lo (fp8)
    Wla = np.vstack([W_l, W_r]) * np.abs(att)[None, :]  # [128f, HC]
    Wp = Wla[:, ch_perm].reshape(128, H, 8, 128)
    hi = np.ascontiguousarray(
        Wp[:, :, 0:4, :].reshape(128, NGH, 128)
    )
    lo = np.ascontiguousarray(
        Wp[:, :, 4:8, :].reshape(128, NGL, 128)
    ) * SW8
    wcat_b = hi.astype(bf16)
    wcat8_b = np.ascontiguousarray(
        lo.reshape(2, 64, NGL, 128).transpose(1, 0, 2, 3)
    ).astype(fp8)  # [f64, ktile(l/r), g, c]
    sgn_p = np.sign(att)[ch_perm].reshape(H, 8, 128)
    selv = np.zeros((128, 2, NPAIR, H), np.float32)
    for g in range(NG):
        hh, j = g // 8, g % 8
        selv[:, g % 2, g // 2, hh] = sgn_p[hh, j]
    sel8 = selv.astype(fp8)

    wl_b = np.ascontiguousarray(
        W_l.reshape(64, NG, 128)
    ).astype(bf16)  # [f, g, c]
    wout_b = np.ascontiguousarray(
        W_out.reshape(NG, 128, N_CLASS).transpose(1, 0, 2)
    ).astype(bf16)  # [p, g, 20]
    bias_sb = np.ascontiguousarray(bias_gat.reshape(NG, 128).T).astype(np.float32)

    # per-node logit scalars
    ul = np.einsum("fhc,hc->fh", W_l.reshape(F_IN, H, C), att.reshape(H, C))
    ur = np.einsum("fhc,hc->fh", W_r.reshape(F_IN, H, C), att.reshape(H, C))
    sl = x @ ul
    sr = x @ ur

    # stable sort edges by destination tile
    dtile = newid[dst] // NT
    order = np.argsort(dtile, kind="stable")
    src_s, dst_s = src[order], dst[order]
    dtile_s = dtile[order]

    in_maps = []
    for core in range(N_CORES):
        xcat = np.zeros((128, EPC), np.float32)
        xcat8 = np.zeros((64, 2, EPC), np.float32)
        ebT = np.zeros((32, EPC), np.float32)
        s01t = np.zeros((128, NSUB, NT), np.float32)
        s01 = np.zeros((NT, EPC), np.float32)
        xsrc = np.zeros((128, NSUB, F_IN), np.float32)
        b0 = 0
        for slot in range(TILES_PER_CORE):
            gt = core * TILES_PER_CORE + tile_order[core][slot]
            idx = np.nonzero(dtile_s == gt)[0]
            ne = len(idx)
            assert ne <= S[slot] * 128, f"tile {gt}: {ne} > {S[slot] * 128}"
            es, ed = src_s[idx], dst_s[idx]
            eslot = np.arange(ne)
            e_abs = b0 * 128 + eslot
            p = eslot % 128
            s_sub = b0 + eslot // 128
            xcat[0:64, e_abs] = x[es].T
            xcat[64:128, e_abs] = x[ed].T
            xcat8[:, 0, e_abs] = x[es].T
            xcat8[:, 1, e_abs] = x[ed].T
            ebT[0:16, e_abs] = np.exp(sl[es] + sr[ed]).T
            ldst = newid[ed] - gt * NT
            s01t[p, s_sub, ldst] = 1.0
            s01[ldst, e_abs] = 1.0
            xsrc[p, s_sub, :] = x[es]
            b0 += S[slot]
        pad = s01.sum(axis=0) == 0.0
        s01[0, pad] = 1.0
        in_maps.append(
            {
                "xcat": xcat.astype(bf16),
                "xcat8": xcat8.astype(fp8),
                "wcat": wcat_b,
                "wcat8": wcat8_b,
                "sel": sel8,
                "ebT": ebT.astype(bf16),
                "s01t": s01t.astype(bf16),
                "s01": np.ascontiguousarray(s01).astype(bf16),
                "xsrc": xsrc.astype(bf16),
                "wl": wl_b,
                "wout": wout_b,
                "bias": bias_sb,
            }
        )
    meta = {"S": S, "perm": perm, "tile_order": tile_order}
    return in_maps, meta


def kernel(**inputs):
    from concourse.bass_utils import run_bass_kernel_spmd

    in_maps, meta = _prep_inputs(**inputs)
    S = meta["S"]
    key = tuple(S)
    if key not in _CACHE:
        act_mask = [m % 2 == 0 for m in range(NPAIR)]
        relu_act_mask = [True for _ in range(NG)]
        _CACHE[key] = _build_nc(S, act_mask, relu_act_mask)
    nc = _CACHE[key]

    res = run_bass_kernel_spmd(nc, in_maps, list(range(N_CORES)))
    b_out = np.asarray(inputs["b_out"], np.float32)
    perm = meta["perm"]
    tile_order = meta["tile_order"]
    z = np.empty((N_NODES, N_CLASS), np.float32)
    for core in range(N_CORES):
        zc = np.asarray(res.results[core]["z"], np.float32)  # [20, 512]
        for slot in range(TILES_PER_CORE):
            gt = core * TILES_PER_CORE + tile_order[core][slot]
            orig_nodes = perm[gt * NT : (gt + 1) * NT]
            z[orig_nodes] = zc[:, slot * NT : (slot + 1) * NT].T
    return z + b_out
